# revision 58
# baseline (speedup 1.0000x reference)
"""Trainium2 Bass kernel for ErnieImageAttention (non-causal MHA with per-head
RMSNorm on q/k + rotary embedding), tensor-parallel over heads on 8 NeuronCores.

Sharding: 16 heads / 8 cores = 2 heads per core. Each core computes its heads'
q/k/v projections, attention, and a partial output projection (row-parallel
Wo); the host sums the 8 partials and adds the bias.

Per-core dataflow (S=4096, D=2048, Hd=128, 2 local heads):
  warmup: input DMAs prioritized across the 3 DMA-capable queues (ht0 sliced
    + chunk-0 weights first, wo deferred into the loop); dummy matmuls keep
    the PE HAM activity monitor warm (full 2.4 GHz clock) through the ~12us
    startup DMA wall; ACT tables preload so table switches stay off the
    critical path.
  phase 1 (per 128-row s-tile):
    qkv = hiddenT-chunk matmuls (bf16) into split ppqk/ppv PSUM tiles
    (1-bank each, so 8 banks cover ppqk x3 + ppv + ptp x2 + the overlap's
    score region); RMSNorm stats via Square-with-accum on ACT; RoPE via
    host-precomputed coefficient tables operating q|k fused at full 512-col
    width (the rsqrt scale applied last, per 128-col head block, casting to
    bf16); PE-transpose q/k 128x128 bf16 tiles into [d, s] layout; v [s, d].
  boundary: the first q-block's head-0 score groups + exps are emitted after
    the s-tile loop; they fill the PE while the last tiles' rope chain
    drains on DVE/ACT. Their first Exp is data-gated (zero bias from the
    last tile's stats) behind phase 1's final Sqrt -- Exp and Sqrt live in
    different ACT table sets and each interleave costs a 1.3us reload.
  phase 2 (per 512-col q-block, per head, k in groups of 2 tiles):
    scoresT[k,q] = kT.T @ qT (bf16) into a 2+2 bank PSUM ping-pong per head;
    one wide Exp per group on ACT. Phase 2 is ACT-exp-bound: 256 exps of
    [128,1024] at (N+352)/1.2ns pace the 2.24us/group steady state, with PE
    (scores+PV+out-proj) and DVE (denominator accA adds + evictions) both
    ~98% occupied right behind it.
    denominator: folded into accA lanes by DVE bf16 adds; the deferred tail
    does 2 ones-matmuls + reciprocal + rank-1 bf16 broadcast into a psF bank
    attn_T[d,q] = sum_k V[k,d]^T expT[k,q] accumulated in PSUM (bf16);
    po is evicted to SBUF bf16 by DVE right after the k-loop
  phase 3 (inline per q-block): fin[s, :2048] = sum_h attnT_h.T @ WoT_h, all
    bf16 (fp32 matmuls run ~2.7x slower on PE); psF has 2 rotating banks so
    out-proj chunks pipeline instead of serializing on one bank; the final
    drain interleaves dummy matmuls so the HAM never rethrottles the clock.

Softmax is max-subtraction-free: logits are ~N(0,1) by construction
(RMSNorm'd q/k, 1/sqrt(Hd) folded into q's rope tables).
"""

import numpy as np
import ml_dtypes

import concourse.bass as bass
import concourse.tile as tile
from concourse import bacc, mybir
from concourse import bass_utils
from concourse.masks import make_identity

F32 = mybir.dt.float32
F32R = mybir.dt.float32r
BF16 = mybir.dt.bfloat16
AX = mybir.AxisListType
AF = mybir.ActivationFunctionType

S = 4096
D = 2048
HD = 128
HEADS = 16
NCORES = 8
HLOC = HEADS // NCORES  # 2 heads per core
DLOC = HLOC * HD  # 256 local head dims
NQKV = 3 * DLOC  # fused q|k|v projection width
CH = D // 128  # 16 contraction chunks for projections
EPS = 1e-5
SCL = 1.0 / np.sqrt(HD)

QCOLS = 512  # q columns per attention block
NWARM0 = 5  # startup HAM-warming dummy matmuls
NWARM1 = 14  # post-tile-0 dummies bridging the tail of the weight-DMA wall
NOVERLAP = 4  # qb0/h0 score groups overlapped into the phase-1 tail


def build(nc, tc, io, s_len):
    st_n = s_len // 128  # s tiles
    qb_n = s_len // QCOLS  # q blocks
    qb_st = QCOLS // 128  # s tiles per q block
    kt_n = st_n  # k tiles

    ht, wqkv, wo, cf, out = (
        io["ht"], io["wqkv"], io["wo"], io["cf"], io["out"],
    )

    import contextlib

    with contextlib.ExitStack() as ctx:
        ctx.enter_context(nc.allow_low_precision(
            reason="bf16/f32r compute; values are O(1) and the rel-err "
                   "budget is 2e-2"))
        consts = ctx.enter_context(tc.tile_pool(name="consts", bufs=1))
        persist = ctx.enter_context(tc.tile_pool(name="persist", bufs=1))
        ht_pool = ctx.enter_context(tc.tile_pool(name="ht", bufs=3))
        cs_pool = ctx.enter_context(tc.tile_pool(name="cs", bufs=3))
        work = ctx.enter_context(tc.tile_pool(name="work", bufs=2))
        et_pool = ctx.enter_context(tc.tile_pool(name="et", bufs=10))
        at_pool = ctx.enter_context(tc.tile_pool(name="at", bufs=6))
        araw_pool = ctx.enter_context(tc.tile_pool(name="araw", bufs=3))
        acc_pool = ctx.enter_context(tc.tile_pool(name="acc", bufs=2))
        rc_pool = ctx.enter_context(tc.tile_pool(name="rc", bufs=2))
        fin_pool = ctx.enter_context(tc.tile_pool(name="fin", bufs=6))

        # startup DMAs first on their queues, prioritized so the first real
        # matmul's inputs (ht0 chunk 0 + weight chunk 0) finish first: ht0
        # arrives in 4 slices on the SP ring, the weight parts fan out over
        # the gpsimd/scalar rings, and wo (not needed until the first
        # out-projection at ~40% of the kernel) is deferred into the loop.
        ht0_t = ht_pool.tile([128, CH, 128], BF16, tag="ht", name="ht0")
        for q in range(2):
            nc.sync.dma_start(out=ht0_t[:, 8 * q:8 * (q + 1), :],
                              in_=ht[0][:, 8 * q:8 * (q + 1), :])
        cf0_t = cs_pool.tile([128, 2, 512], BF16, tag="cf", name="cf0")
        nc.scalar.dma_start(out=cf0_t[:], in_=cf[0:128, :, :])

        w_parts = [(0, 1, nc.gpsimd), (1, 5, nc.gpsimd), (6, 5, nc.scalar),
                   (11, 5, nc.sync)]
        w_sb = [None] * CH  # per-chunk views
        for pi, (c0, ln, eng) in enumerate(w_parts):
            t = consts.tile([128, ln, NQKV], BF16, name=f"wqkv{pi}",
                            tag=f"wqkv{pi}")
            eng.dma_start(out=t[:], in_=wqkv[:, c0:c0 + ln, :])
            for j in range(ln):
                w_sb[c0 + j] = t[:, j, :]
        wo_sb = consts.tile([128, HLOC, D], BF16)

        # constants (warmup scratch first so the dummy matmuls start early)
        wdum = consts.tile([128, 512], BF16)
        nc.vector.memset(wdum[:], 0.0)
        ident = consts.tile([128, 128], BF16)
        make_identity(nc, ident[:])
        ones_f32 = consts.tile([128, 1], F32)
        nc.vector.memset(ones_f32[:], 1.0)
        ones_col_bf = consts.tile([128, 1], BF16)
        nc.vector.tensor_copy(ones_col_bf[:], ones_f32[:])
        ones_row_f32 = consts.tile([1, 128], F32)
        nc.vector.memset(ones_row_f32[:], 1.0)
        ones_row = consts.tile([1, 128], BF16)
        nc.vector.tensor_copy(ones_row[:], ones_row_f32[:])
        eps_t = consts.tile([128, 1], F32)
        nc.vector.memset(eps_t[:], EPS)

        # ACT table preloads (exp first so its set sits in slot 0 for all of
        # phase 2; sqrt's set lands in slot 1 for phase 1)
        wsc = consts.tile([128, 1], F32)
        nc.vector.memset(wsc[:], 1.0)
        wout = consts.tile([128, 1], F32)
        nc.scalar.activation(wout[:], wsc[:], AF.Exp)
        nc.scalar.activation(wout[:], wsc[:], AF.Square)
        nc.scalar.activation(wout[:], wsc[:], AF.Sqrt, bias=eps_t[:],
                             scale=1.0 / HD)
        nc.scalar.copy(wout[:], wsc[:])

        # persistent per-head transposed q/k and v
        qT_sb = persist.tile([128, HLOC, st_n, 128], BF16)
        kT_sb = persist.tile([128, HLOC, st_n, 128], BF16)
        v_sb = persist.tile([128, st_n, DLOC], BF16)

        KG = 2  # k tiles per score group
        kg_n = kt_n // KG
        ov_ets = {}  # (h, g) -> et tile, prepopulated by the overlap

        def score_group(pool, h, g, q_rhs_h, bias=0.0, split_exp=False):
            sc = pool.tile([128, KG, QCOLS], F32, tag=f"sc{h}",
                           name=f"sc{h}")
            for j in range(KG):
                nc.tensor.matmul(sc[:, j, :], kT_sb[:, h, g * KG + j, :],
                                 q_rhs_h, start=True, stop=True)
            et = et_pool.tile([128, KG, QCOLS], BF16, tag="et")
            if split_exp:
                # per-k-tile exps: subtile WAR lets the next group's first
                # matmul start as soon as the matching half is consumed
                for j in range(KG):
                    nc.scalar.activation(et[:, j, :], sc[:, j, :], AF.Exp,
                                         bias=bias)
            else:
                nc.scalar.activation(et[:], sc[:], AF.Exp, bias=bias)
            return et

        # ---------------- phase 1: projections + norm + rope + transpose ----
        # psS0 holds one single-k-tile score bank so the first q-block's
        # score matmuls can overlap the phase-1 tail (the PE would otherwise
        # idle while the last s-tiles' rope drains on DVE); it doubles as the
        # target for HAM-warming dummy matmuls during the startup DMA wait.
        with (
            tc.tile_pool(name="psS0", bufs=1, space="PSUM") as psS0,
            tc.tile_pool(name="ps1", bufs=1, space="PSUM") as ps1,
        ):
            dum = psS0.tile([128, KG, QCOLS], F32, tag="sc0", name="dum")
            for i in range(NWARM0):
                nc.tensor.matmul(dum[:, 0, :], wdum[:, 0:128], wdum[:],
                                 start=(i == 0), stop=False)

            for st in range(st_n):
                if st == 0:
                    ht_t = ht0_t
                    cf_t = cf0_t
                else:
                    ss = slice(st * 128, (st + 1) * 128)
                    ht_t = ht_pool.tile([128, CH, 128], BF16, tag="ht")
                    nc.sync.dma_start(out=ht_t[:], in_=ht[st])
                    cf_t = cs_pool.tile([128, 2, 512], BF16, tag="cf")
                    nc.sync.dma_start(out=cf_t[:], in_=cf[ss, :, :])
                if st == 6:
                    # wo fetch deferred off the startup critical path
                    nc.scalar.dma_start(out=wo_sb[:], in_=wo)

                ppqk = ps1.tile([128, 512], F32, tag="ppqk", bufs=3)
                ppv = ps1.tile([128, 256], F32, tag="ppv", bufs=1)
                for c in range(CH):
                    lhs = ht_t[:, c, :]
                    # moving operand limit is 512 elements: q|k then v
                    nc.tensor.matmul(ppqk[:], lhs, w_sb[c][:, 0:512],
                                     start=(c == 0), stop=(c == CH - 1))
                    nc.tensor.matmul(ppv[:], lhs, w_sb[c][:, 512:NQKV],
                                     start=(c == 0), stop=(c == CH - 1))
                    if st == 0:
                        # tile 0 is weight-DMA-supply-limited: dummy matmuls
                        # between chunks keep the PE HAM busy-window alive
                        nc.tensor.matmul(dum[:, 0, :], wdum[:, 0:128],
                                         wdum[:], start=False, stop=False)

                if st == 0:
                    # ~5us of DMA wall remains after tile 0; bridge most of
                    # it (under-budgeted so real work is never displaced)
                    for i in range(NWARM1):
                        nc.tensor.matmul(dum[:, 0, :], wdum[:, 0:128],
                                         wdum[:], start=False,
                                         stop=(i == NWARM1 - 1))

                # v: PSUM -> SBUF bf16 (first ACT op of the tile so the
                # single-buffered ppv bank frees before the next tile)
                nc.scalar.copy(v_sb[:, st, :], ppv[:])

                # rms stats: Square with free-dim accumulation -> sum(x^2)
                # per 128-col block (q0, q1, k0, k1)
                var4 = work.tile([128, 4], F32, tag="var4")
                sqd = work.tile([128, HD], F32, tag="sqd")  # dump
                for j in range(4):
                    nc.scalar.activation(sqd[:],
                                         ppqk[:, j * 128:(j + 1) * 128],
                                         AF.Square,
                                         accum_out=var4[:, j:j + 1])
                sig4 = work.tile([128, 4], F32, tag="sig4")
                nc.scalar.activation(sig4[:], var4[:], AF.Sqrt,
                                     bias=eps_t[:], scale=1.0 / HD)
                r4 = work.tile([128, 4], F32, tag="r4")
                nc.vector.reciprocal_approx_fast(r4[:], sig4[:])

                # rope on the fused q|k 512-col block:
                #   xf = x . CG + shift64(x) . SG ; xa = r * xf (per head blk)
                pqk = ppqk[:]
                m1 = work.tile([128, 512], F32, tag="m1")
                nc.vector.tensor_mul(m1[:], pqk, cf_t[:, 0, :])
                m2 = work.tile([128, 512], F32, tag="m2")
                x4 = pqk.rearrange("p (h t u) -> p h t u", h=4, t=2)
                m4 = m2[:].rearrange("p (h t u) -> p h t u", h=4, t=2)
                g4 = cf_t[:, 1, :].rearrange("p (h t u) -> p h t u", h=4, t=2)
                nc.vector.tensor_mul(m4[:, :, 0, :], x4[:, :, 1, :],
                                     g4[:, :, 0, :])
                nc.vector.tensor_mul(m4[:, :, 1, :], x4[:, :, 0, :],
                                     g4[:, :, 1, :])
                xf = work.tile([128, 512], F32, tag="xf")
                nc.vector.tensor_add(xf[:], m1[:], m2[:])
                xa = work.tile([128, 512], BF16, tag="xa")
                for j in range(4):
                    js = slice(j * 128, (j + 1) * 128)
                    nc.vector.tensor_scalar_mul(xa[:, js], xf[:, js],
                                                r4[:, j:j + 1])
                # blocks: 0,1 -> qT heads; 2,3 -> kT heads. (PE transpose +
                # ACT eviction; DMA XBAR transposes measured far slower --
                # ~1.2us queue time per trigger plus a slow transfer path.)
                for j in range(4):
                    dstT, h = (qT_sb, j) if j < 2 else (kT_sb, j - 2)
                    ptp = ps1.tile([128, 128], BF16, tag="ptp", bufs=2)
                    nc.tensor.transpose(ptp[:], xa[:, j * 128:(j + 1) * 128],
                                        ident[:])
                    nc.scalar.copy(dstT[:, h, st, :], ptp[:])

            # overlap: the first q-block's head-0 score groups fill the PE
            # while the last s-tiles' rope chain drains on DVE/ACT. The
            # first exp carries a zero bias derived from the last tile's r4
            # so the scheduler cannot hoist it before phase 1's final Sqrt
            # (Exp and Sqrt live in different ACT table sets; an interleave
            # costs a 1.3us table reload per switch).
            zb = work.tile([128, 1], F32, tag="zb")
            nc.vector.tensor_scalar_mul(zb[:], sig4[:, 0:1], 0.0)
            q0 = qT_sb[:, 0, 0:qb_st, :]
            for g in range(NOVERLAP):
                ov_ets[(0, g)] = score_group(psS0, 0, g, q0,
                                             bias=zb[:] if g == 0 else 0.0,
                                             split_exp=True)

        # ---------------- phase 2+3: attention + output projection ----------
        # The two heads' score->exp->PV chains are interleaved step by step:
        # while ACT runs one head's exp, PE runs the other head's matmuls, so
        # every cross-engine dependency has a full step (~2.3us) of slack and
        # semaphore latency never stalls the in-order PE queue.
        # PSUM: sc_h0(2) + sc_h1(2) + po_h0(1) + po_h1(1) + pf(2, shared with
        # the tails' pd/broadcast bank) = 8.
        with (
            tc.tile_pool(name="psS", bufs=1, space="PSUM") as psS,
            tc.tile_pool(name="psP", bufs=1, space="PSUM") as psP,
            tc.tile_pool(name="psF", bufs=2, space="PSUM") as psF,
        ):
            pending = []  # deferred tails + out-proj chunks

            def outproj_chunks(qb, ats):
                chunks = []
                for sti in range(qb_st):
                    st = qb * qb_st + sti
                    sl = slice(sti * 128, (sti + 1) * 128)
                    for nchunk in range(D // 512):
                        ns = slice(nchunk * 512, (nchunk + 1) * 512)

                        def emit(st=st, sl=sl, ns=ns, ats=ats):
                            pf = psF.tile([128, QCOLS], F32, tag="pf",
                                          name=f"pf_{st}_{ns.start}")
                            for h in range(HLOC):
                                nc.tensor.matmul(pf[:], ats[h][:, sl],
                                                 wo_sb[:, h, ns],
                                                 start=(h == 0),
                                                 stop=(h == HLOC - 1))
                            fin = fin_pool.tile([128, 512], F32, tag="fin")
                            nc.vector.tensor_copy(fin[:], pf[:])
                            nc.sync.dma_start(
                                out=out[st * 128:(st + 1) * 128, ns],
                                in_=fin[:])
                        chunks.append(emit)
                return chunks

            for qb in range(qb_n):
                q_rhs = [qT_sb[:, h, qb * qb_st:(qb + 1) * qb_st, :]
                         for h in range(HLOC)]
                accA = [acc_pool.tile([128, KG, QCOLS], BF16,
                                      tag=f"accA{h}", name=f"accA{h}")
                        for h in range(HLOC)]
                po = [psP.tile([128, QCOLS], F32, tag=f"po{h}",
                               name=f"po{h}") for h in range(HLOC)]
                ets = dict(ov_ets) if qb == 0 else {}

                def flush(h, g):
                    et = ets.pop((h, g))
                    acc = accA[h][:, :, :]
                    if g == 0:
                        nc.vector.tensor_copy(acc, et[:])
                    else:
                        nc.vector.tensor_add(acc, acc, et[:])
                    for j in range(KG):
                        kt = g * KG + j
                        nc.tensor.matmul(po[h][:],
                                         v_sb[:, kt, h * HD:(h + 1) * HD],
                                         et[:, j, :], start=(kt == 0),
                                         stop=(kt == kt_n - 1))

                for g in range(kg_n):
                    for h in range(HLOC):
                        if (h, g) in ets:
                            continue  # pre-computed in the phase-1 overlap
                        ets[(h, g)] = score_group(psS, h, g, q_rhs[h])
                    if g >= 1:
                        for h in range(HLOC):
                            flush(h, g - 1)
                    if pending:
                        pending.pop(0)()
                    if g % 8 == 7 and pending:
                        pending.pop(0)()
                for h in range(HLOC):
                    flush(h, kg_n - 1)

                # evict po now (DVE) so next qb's PV chains don't wait on the
                # deferred tails; ACT's FIFO stays exps-only
                ats = []
                for h in range(HLOC):
                    araw = araw_pool.tile([128, QCOLS], BF16, tag="araw")
                    nc.vector.tensor_copy(araw[:], po[h][:])
                    at = at_pool.tile([128, QCOLS], BF16, tag="at")
                    ats.append(at)

                    def tail(accA=accA[h], araw=araw, at=at):
                        pdb = psF.tile([128, QCOLS], F32, tag="pf",
                                       name="pdb")
                        pd = pdb[0:1, :]
                        for i in range(KG):
                            nc.tensor.matmul(pd, ones_col_bf[:],
                                             accA[:, i, :],
                                             start=(i == 0),
                                             stop=(i == KG - 1))
                        rsb = rc_pool.tile([1, QCOLS], F32, tag="rsb")
                        nc.vector.reciprocal_approx_fast(rsb[:], pd)
                        rsr = rc_pool.tile([1, QCOLS], BF16, tag="rsr")
                        nc.vector.tensor_copy(rsr[:], rsb[:])
                        nc.tensor.matmul(pdb[:], ones_row[:], rsr[:],
                                         start=True, stop=True)
                        nc.vector.tensor_mul(at[:], araw[:], pdb[:])
                    pending.insert(h, tail)

                pending += outproj_chunks(qb, ats)

            # final drain: the last q-block's tails + out-proj chunks have no
            # later k-loop to interleave into, so the PE runs at ~50% duty
            # here; dummy matmuls keep the HAM activity window busy (a
            # re-throttle would halve the PE clock for the whole drain)
            dum2 = psS.tile([128, KG, QCOLS], F32, tag="sc0", name="dum2")
            ndum = 2 * len(pending)
            di = 0
            while pending:
                pending.pop(0)()
                for _ in range(2):
                    nc.tensor.matmul(dum2[:, 0, :], wdum[:, 0:128], wdum[:],
                                     start=(di == 0), stop=(di == ndum - 1))
                    di += 1


def build_program(s_len=S):
    nc = bacc.Bacc("TRN2", target_bir_lowering=False, debug=False,
                   enable_asserts=False)
    st_n = s_len // 128
    io = {
        "ht": nc.dram_tensor("ht", [st_n, 128, CH, 128], BF16,
                             kind="ExternalInput").ap(),
        "wqkv": nc.dram_tensor("wqkv", [128, CH, NQKV], BF16,
                               kind="ExternalInput").ap(),
        "wo": nc.dram_tensor("wo", [128, HLOC, D], BF16,
                             kind="ExternalInput").ap(),
        "cf": nc.dram_tensor("cf", [s_len, 2, 512], BF16,
                             kind="ExternalInput").ap(),
        "out": nc.dram_tensor("out", [s_len, D], F32,
                              kind="ExternalOutput").ap(),
    }
    with tile.TileContext(nc) as tc:
        build(nc, tc, io, s_len)
    nc.compile()
    return nc


def prep_inputs(inputs, s_len=S):
    """Host-side preprocessing: transposed/tiled bf16 layouts + rope
    coefficient tables (g gains and the 1/sqrt(Hd) scale folded in,
    duplicated per local head and fused q|k for full-width elementwise
    ops)."""
    bf16 = ml_dtypes.bfloat16
    hs = np.asarray(inputs["hidden_states"], np.float32).reshape(s_len, D)
    st_n = s_len // 128
    ht = np.ascontiguousarray(
        hs.reshape(st_n, 128, CH, 128).transpose(0, 3, 2, 1)).astype(bf16)

    fc = np.asarray(inputs["freqs_cis"], np.float32).reshape(s_len, HD)
    cos = np.cos(fc)
    sin = np.sin(fc)
    gq = np.asarray(inputs["gq"], np.float32)
    gk = np.asarray(inputs["gk"], np.float32)

    def coef(g, scale):
        cg = cos * g[None, :] * scale
        sg = np.empty_like(sin)
        sg[:, :64] = -sin[:, :64] * g[None, 64:] * scale
        sg[:, 64:] = sin[:, 64:] * g[None, :64] * scale
        return np.tile(cg, (1, HLOC)), np.tile(sg, (1, HLOC))

    cgq, sgq = coef(gq, SCL)
    cgk, sgk = coef(gk, 1.0)
    cgqk = np.concatenate([cgq, cgk], axis=1)
    sgqk = np.concatenate([sgq, sgk], axis=1)
    cf = np.ascontiguousarray(
        np.stack([cgqk, sgqk], axis=1)).astype(bf16)

    Wq = np.asarray(inputs["Wq"], np.float32)
    Wk = np.asarray(inputs["Wk"], np.float32)
    Wv = np.asarray(inputs["Wv"], np.float32)
    Wo = np.asarray(inputs["Wo"], np.float32)

    in_maps = []
    for c in range(NCORES):
        cols = slice(DLOC * c, DLOC * (c + 1))

        def wtile(W):
            # [D, DLOC] -> [128(part), CH, DLOC]
            return W[cols, :].T.reshape(CH, 128, DLOC).transpose(1, 0, 2)

        wqkv_c = np.ascontiguousarray(
            np.concatenate([wtile(Wq), wtile(Wk), wtile(Wv)],
                           axis=2)).astype(bf16)
        wo_c = np.ascontiguousarray(
            Wo[:, cols].T.reshape(HLOC, 128, D).transpose(1, 0, 2)
        ).astype(bf16)
        in_maps.append({
            "ht": ht, "wqkv": wqkv_c, "wo": wo_c, "cf": cf,
        })
    return in_maps


_CACHE = {}


def run_full(inputs, trace=False, **kw):
    if "nc" not in _CACHE:
        _CACHE["nc"] = build_program(S)
    nc = _CACHE["nc"]
    in_maps = prep_inputs(inputs, S)
    res = bass_utils.run_bass_kernel_spmd(
        nc, in_maps, core_ids=list(range(NCORES)), trace=trace, **kw)
    total = res.results[0]["out"].astype(np.float64)
    for c in range(1, NCORES):
        total += res.results[c]["out"]
    total += np.asarray(inputs["bo"], np.float64)[None, :]
    out = total.astype(np.float32).reshape(1, S, D)
    return out, res


def kernel(**inputs):
    out, _ = run_full(inputs, trace=False)
    return out


# revision 60
# speedup vs baseline: 1.0004x; 1.0004x over previous
"""Trainium2 Bass kernel for ErnieImageAttention (non-causal MHA with per-head
RMSNorm on q/k + rotary embedding), tensor-parallel over heads on 8 NeuronCores.

Sharding: 16 heads / 8 cores = 2 heads per core. Each core computes its heads'
q/k/v projections, attention, and a partial output projection (row-parallel
Wo); the host sums the 8 partials and adds the bias.

Per-core dataflow (S=4096, D=2048, Hd=128, 2 local heads):
  warmup: input DMAs prioritized across the 3 DMA-capable queues (ht0 sliced
    + chunk-0 weights first, wo deferred into the loop); dummy matmuls keep
    the PE HAM activity monitor warm (full 2.4 GHz clock) through the ~12us
    startup DMA wall; ACT tables preload so table switches stay off the
    critical path.
  phase 1 (per 128-row s-tile):
    qkv = hiddenT-chunk matmuls (bf16) into split ppqk/ppv PSUM tiles
    (1-bank each, so 8 banks cover ppqk x3 + ppv + ptp x2 + the overlap's
    score region); RMSNorm stats via Square-with-accum on ACT; RoPE via
    host-precomputed coefficient tables operating q|k fused at full 512-col
    width (the rsqrt scale applied last, per 128-col head block, casting to
    bf16); PE-transpose q/k 128x128 bf16 tiles into [d, s] layout; v [s, d].
  boundary: the first q-block's head-0 score groups + exps are emitted after
    the s-tile loop; they fill the PE while the last tiles' rope chain
    drains on DVE/ACT. Their first Exp is data-gated (zero bias from the
    last tile's stats) behind phase 1's final Sqrt -- Exp and Sqrt live in
    different ACT table sets and each interleave costs a 1.3us reload.
  phase 2 (per 512-col q-block, per head, k in groups of 2 tiles):
    scoresT[k,q] = kT.T @ qT (bf16) into a 2+2 bank PSUM ping-pong per head;
    one wide Exp per group on ACT. Phase 2 is ACT-exp-bound: 256 exps of
    [128,1024] at (N+352)/1.2ns pace the 2.24us/group steady state, with PE
    (scores+PV+out-proj) and DVE (denominator accA adds + evictions) both
    ~98% occupied right behind it.
    denominator: folded into accA lanes by DVE bf16 adds; the deferred tail
    does 2 ones-matmuls + reciprocal + rank-1 bf16 broadcast into a psF bank
    attn_T[d,q] = sum_k V[k,d]^T expT[k,q] accumulated in PSUM (bf16);
    po is evicted to SBUF bf16 by DVE right after the k-loop
  phase 3 (inline per q-block): fin[s, :2048] = sum_h attnT_h.T @ WoT_h, all
    bf16 (fp32 matmuls run ~2.7x slower on PE); psF has 2 rotating banks so
    out-proj chunks pipeline instead of serializing on one bank; the final
    drain interleaves dummy matmuls so the HAM never rethrottles the clock.

Softmax is max-subtraction-free: logits are ~N(0,1) by construction
(RMSNorm'd q/k, 1/sqrt(Hd) folded into q's rope tables).
"""

import numpy as np
import ml_dtypes

import concourse.bass as bass
import concourse.tile as tile
from concourse import bacc, mybir
from concourse import bass_utils
from concourse.masks import make_identity

F32 = mybir.dt.float32
F32R = mybir.dt.float32r
BF16 = mybir.dt.bfloat16
AX = mybir.AxisListType
AF = mybir.ActivationFunctionType

S = 4096
D = 2048
HD = 128
HEADS = 16
NCORES = 8
HLOC = HEADS // NCORES  # 2 heads per core
DLOC = HLOC * HD  # 256 local head dims
NQKV = 3 * DLOC  # fused q|k|v projection width
CH = D // 128  # 16 contraction chunks for projections
EPS = 1e-5
SCL = 1.0 / np.sqrt(HD)

QCOLS = 512  # q columns per attention block
NWARM0 = 10  # startup HAM-warming dummy matmuls
NWARM1 = 14  # post-tile-0 dummies bridging the tail of the weight-DMA wall
NWARMB = 10  # dummies filling the last tile's rope-wait before phase 2
NOVERLAP = 4  # qb0/h0 score groups overlapped into the phase-1 tail


def build(nc, tc, io, s_len):
    st_n = s_len // 128  # s tiles
    qb_n = s_len // QCOLS  # q blocks
    qb_st = QCOLS // 128  # s tiles per q block
    kt_n = st_n  # k tiles

    ht, wqkv, wo, cf, out = (
        io["ht"], io["wqkv"], io["wo"], io["cf"], io["out"],
    )

    import contextlib

    with contextlib.ExitStack() as ctx:
        ctx.enter_context(nc.allow_low_precision(
            reason="bf16/f32r compute; values are O(1) and the rel-err "
                   "budget is 2e-2"))
        consts = ctx.enter_context(tc.tile_pool(name="consts", bufs=1))
        persist = ctx.enter_context(tc.tile_pool(name="persist", bufs=1))
        ht_pool = ctx.enter_context(tc.tile_pool(name="ht", bufs=3))
        cs_pool = ctx.enter_context(tc.tile_pool(name="cs", bufs=3))
        work = ctx.enter_context(tc.tile_pool(name="work", bufs=2))
        et_pool = ctx.enter_context(tc.tile_pool(name="et", bufs=10))
        at_pool = ctx.enter_context(tc.tile_pool(name="at", bufs=6))
        araw_pool = ctx.enter_context(tc.tile_pool(name="araw", bufs=3))
        acc_pool = ctx.enter_context(tc.tile_pool(name="acc", bufs=2))
        rc_pool = ctx.enter_context(tc.tile_pool(name="rc", bufs=2))
        fin_pool = ctx.enter_context(tc.tile_pool(name="fin", bufs=6))

        # startup DMAs first on their queues, prioritized so the first real
        # matmul's inputs (ht0 chunk 0 + weight chunk 0) finish first: ht0
        # arrives in 4 slices on the SP ring, the weight parts fan out over
        # the gpsimd/scalar rings, and wo (not needed until the first
        # out-projection at ~40% of the kernel) is deferred into the loop.
        ht0_t = ht_pool.tile([128, CH, 128], BF16, tag="ht", name="ht0")
        for q in range(2):
            nc.sync.dma_start(out=ht0_t[:, 8 * q:8 * (q + 1), :],
                              in_=ht[0][:, 8 * q:8 * (q + 1), :])
        cf0_t = cs_pool.tile([128, 2, 512], BF16, tag="cf", name="cf0")
        nc.scalar.dma_start(out=cf0_t[:], in_=cf[0:128, :, :])

        w_parts = [(0, 1, nc.gpsimd), (1, 5, nc.gpsimd), (6, 5, nc.scalar),
                   (11, 5, nc.sync)]
        w_sb = [None] * CH  # per-chunk views
        for pi, (c0, ln, eng) in enumerate(w_parts):
            t = consts.tile([128, ln, NQKV], BF16, name=f"wqkv{pi}",
                            tag=f"wqkv{pi}")
            eng.dma_start(out=t[:], in_=wqkv[:, c0:c0 + ln, :])
            for j in range(ln):
                w_sb[c0 + j] = t[:, j, :]
        wo_sb = consts.tile([128, HLOC, D], BF16)

        # constants (warmup scratch first so the dummy matmuls start early)
        wdum = consts.tile([128, 512], BF16)
        nc.vector.memset(wdum[:], 0.0)
        ident = consts.tile([128, 128], BF16)
        make_identity(nc, ident[:])
        ones_f32 = consts.tile([128, 1], F32)
        nc.vector.memset(ones_f32[:], 1.0)
        ones_col_bf = consts.tile([128, 1], BF16)
        nc.vector.tensor_copy(ones_col_bf[:], ones_f32[:])
        ones_row_f32 = consts.tile([1, 128], F32)
        nc.vector.memset(ones_row_f32[:], 1.0)
        ones_row = consts.tile([1, 128], BF16)
        nc.vector.tensor_copy(ones_row[:], ones_row_f32[:])
        eps_t = consts.tile([128, 1], F32)
        nc.vector.memset(eps_t[:], EPS)

        # ACT table preloads (exp first so its set sits in slot 0 for all of
        # phase 2; sqrt's set lands in slot 1 for phase 1)
        wsc = consts.tile([128, 1], F32)
        nc.vector.memset(wsc[:], 1.0)
        wout = consts.tile([128, 1], F32)
        nc.scalar.activation(wout[:], wsc[:], AF.Exp)
        nc.scalar.activation(wout[:], wsc[:], AF.Square)
        nc.scalar.activation(wout[:], wsc[:], AF.Sqrt, bias=eps_t[:],
                             scale=1.0 / HD)
        nc.scalar.copy(wout[:], wsc[:])

        # persistent per-head transposed q/k and v
        qT_sb = persist.tile([128, HLOC, st_n, 128], BF16)
        kT_sb = persist.tile([128, HLOC, st_n, 128], BF16)
        v_sb = persist.tile([128, st_n, DLOC], BF16)

        KG = 2  # k tiles per score group
        kg_n = kt_n // KG
        ov_ets = {}  # (h, g) -> et tile, prepopulated by the overlap

        def score_group(pool, h, g, q_rhs_h, bias=0.0, split_exp=False):
            sc = pool.tile([128, KG, QCOLS], F32, tag=f"sc{h}",
                           name=f"sc{h}")
            for j in range(KG):
                nc.tensor.matmul(sc[:, j, :], kT_sb[:, h, g * KG + j, :],
                                 q_rhs_h, start=True, stop=True)
            et = et_pool.tile([128, KG, QCOLS], BF16, tag="et")
            if split_exp:
                # per-k-tile exps: subtile WAR lets the next group's first
                # matmul start as soon as the matching half is consumed
                for j in range(KG):
                    nc.scalar.activation(et[:, j, :], sc[:, j, :], AF.Exp,
                                         bias=bias)
            else:
                nc.scalar.activation(et[:], sc[:], AF.Exp, bias=bias)
            return et

        # ---------------- phase 1: projections + norm + rope + transpose ----
        # psS0 holds one single-k-tile score bank so the first q-block's
        # score matmuls can overlap the phase-1 tail (the PE would otherwise
        # idle while the last s-tiles' rope drains on DVE); it doubles as the
        # target for HAM-warming dummy matmuls during the startup DMA wait.
        with (
            tc.tile_pool(name="psS0", bufs=1, space="PSUM") as psS0,
            tc.tile_pool(name="ps1", bufs=1, space="PSUM") as ps1,
        ):
            dum = psS0.tile([128, KG, QCOLS], F32, tag="sc0", name="dum")
            for i in range(NWARM0):
                nc.tensor.matmul(dum[:, 0, :], wdum[:, 0:128], wdum[:],
                                 start=(i == 0), stop=False)

            for st in range(st_n):
                if st == 0:
                    ht_t = ht0_t
                    cf_t = cf0_t
                else:
                    ss = slice(st * 128, (st + 1) * 128)
                    ht_t = ht_pool.tile([128, CH, 128], BF16, tag="ht")
                    nc.sync.dma_start(out=ht_t[:], in_=ht[st])
                    cf_t = cs_pool.tile([128, 2, 512], BF16, tag="cf")
                    nc.sync.dma_start(out=cf_t[:], in_=cf[ss, :, :])
                if st == 6:
                    # wo fetch deferred off the startup critical path
                    nc.scalar.dma_start(out=wo_sb[:], in_=wo)

                ppqk = ps1.tile([128, 512], F32, tag="ppqk", bufs=3)
                ppv = ps1.tile([128, 256], F32, tag="ppv", bufs=1)
                for c in range(CH):
                    lhs = ht_t[:, c, :]
                    # moving operand limit is 512 elements: q|k then v
                    nc.tensor.matmul(ppqk[:], lhs, w_sb[c][:, 0:512],
                                     start=(c == 0), stop=(c == CH - 1))
                    nc.tensor.matmul(ppv[:], lhs, w_sb[c][:, 512:NQKV],
                                     start=(c == 0), stop=(c == CH - 1))
                    if st == 0:
                        # tile 0 is weight-DMA-supply-limited: dummy matmuls
                        # between chunks keep the PE HAM busy-window alive
                        nc.tensor.matmul(dum[:, 0, :], wdum[:, 0:128],
                                         wdum[:], start=False, stop=False)

                if st == 0:
                    # ~5us of DMA wall remains after tile 0; bridge most of
                    # it (under-budgeted so real work is never displaced)
                    for i in range(NWARM1):
                        nc.tensor.matmul(dum[:, 0, :], wdum[:, 0:128],
                                         wdum[:], start=False,
                                         stop=(i == NWARM1 - 1))
                if st == st_n - 1:
                    # the last tile's transposes sit next in the in-order PE
                    # queue but wait ~4us for its rope chain on DVE; these
                    # fill that window (and keep the HAM clock warm into the
                    # phase boundary)
                    dumB = psS0.tile([128, KG, QCOLS], F32, tag="sc0",
                                     name="dumB")
                    for i in range(NWARMB):
                        nc.tensor.matmul(dumB[:, 0, :], wdum[:, 0:128],
                                         wdum[:], start=(i == 0),
                                         stop=(i == NWARMB - 1))

                # v: PSUM -> SBUF bf16 (first ACT op of the tile so the
                # single-buffered ppv bank frees before the next tile)
                nc.scalar.copy(v_sb[:, st, :], ppv[:])

                # rms stats: Square with free-dim accumulation -> sum(x^2)
                # per 128-col block (q0, q1, k0, k1)
                var4 = work.tile([128, 4], F32, tag="var4")
                sqd = work.tile([128, HD], F32, tag="sqd")  # dump
                for j in range(4):
                    nc.scalar.activation(sqd[:],
                                         ppqk[:, j * 128:(j + 1) * 128],
                                         AF.Square,
                                         accum_out=var4[:, j:j + 1])
                sig4 = work.tile([128, 4], F32, tag="sig4")
                nc.scalar.activation(sig4[:], var4[:], AF.Sqrt,
                                     bias=eps_t[:], scale=1.0 / HD)
                r4 = work.tile([128, 4], F32, tag="r4")
                nc.vector.reciprocal_approx_fast(r4[:], sig4[:])

                # rope on the fused q|k 512-col block:
                #   xf = x . CG + shift64(x) . SG ; xa = r * xf (per head blk)
                pqk = ppqk[:]
                m1 = work.tile([128, 512], F32, tag="m1")
                nc.vector.tensor_mul(m1[:], pqk, cf_t[:, 0, :])
                m2 = work.tile([128, 512], F32, tag="m2")
                x4 = pqk.rearrange("p (h t u) -> p h t u", h=4, t=2)
                m4 = m2[:].rearrange("p (h t u) -> p h t u", h=4, t=2)
                g4 = cf_t[:, 1, :].rearrange("p (h t u) -> p h t u", h=4, t=2)
                nc.vector.tensor_mul(m4[:, :, 0, :], x4[:, :, 1, :],
                                     g4[:, :, 0, :])
                nc.vector.tensor_mul(m4[:, :, 1, :], x4[:, :, 0, :],
                                     g4[:, :, 1, :])
                xf = work.tile([128, 512], F32, tag="xf")
                nc.vector.tensor_add(xf[:], m1[:], m2[:])
                xa = work.tile([128, 512], BF16, tag="xa")
                for j in range(4):
                    js = slice(j * 128, (j + 1) * 128)
                    nc.vector.tensor_scalar_mul(xa[:, js], xf[:, js],
                                                r4[:, j:j + 1])
                # blocks: 0,1 -> qT heads; 2,3 -> kT heads. (PE transpose +
                # ACT eviction; DMA XBAR transposes measured far slower --
                # ~1.2us queue time per trigger plus a slow transfer path.)
                for j in range(4):
                    dstT, h = (qT_sb, j) if j < 2 else (kT_sb, j - 2)
                    ptp = ps1.tile([128, 128], BF16, tag="ptp", bufs=2)
                    nc.tensor.transpose(ptp[:], xa[:, j * 128:(j + 1) * 128],
                                        ident[:])
                    nc.scalar.copy(dstT[:, h, st, :], ptp[:])

            # overlap: the first q-block's head-0 score groups fill the PE
            # while the last s-tiles' rope chain drains on DVE/ACT. The
            # first exp carries a zero bias derived from the last tile's r4
            # so the scheduler cannot hoist it before phase 1's final Sqrt
            # (Exp and Sqrt live in different ACT table sets; an interleave
            # costs a 1.3us table reload per switch).
            zb = work.tile([128, 1], F32, tag="zb")
            nc.vector.tensor_scalar_mul(zb[:], sig4[:, 0:1], 0.0)
            q0 = qT_sb[:, 0, 0:qb_st, :]
            for g in range(NOVERLAP):
                ov_ets[(0, g)] = score_group(psS0, 0, g, q0,
                                             bias=zb[:] if g == 0 else 0.0,
                                             split_exp=True)

        # ---------------- phase 2+3: attention + output projection ----------
        # The two heads' score->exp->PV chains are interleaved step by step:
        # while ACT runs one head's exp, PE runs the other head's matmuls, so
        # every cross-engine dependency has a full step (~2.3us) of slack and
        # semaphore latency never stalls the in-order PE queue.
        # PSUM: sc_h0(2) + sc_h1(2) + po_h0(1) + po_h1(1) + pf(2, shared with
        # the tails' pd/broadcast bank) = 8.
        with (
            tc.tile_pool(name="psS", bufs=1, space="PSUM") as psS,
            tc.tile_pool(name="psP", bufs=1, space="PSUM") as psP,
            tc.tile_pool(name="psF", bufs=2, space="PSUM") as psF,
        ):
            pending = []  # deferred tails + out-proj chunks

            def outproj_chunks(qb, ats):
                chunks = []
                for sti in range(qb_st):
                    st = qb * qb_st + sti
                    sl = slice(sti * 128, (sti + 1) * 128)
                    for nchunk in range(D // 512):
                        ns = slice(nchunk * 512, (nchunk + 1) * 512)

                        def emit(st=st, sl=sl, ns=ns, ats=ats):
                            pf = psF.tile([128, QCOLS], F32, tag="pf",
                                          name=f"pf_{st}_{ns.start}")
                            for h in range(HLOC):
                                nc.tensor.matmul(pf[:], ats[h][:, sl],
                                                 wo_sb[:, h, ns],
                                                 start=(h == 0),
                                                 stop=(h == HLOC - 1))
                            fin = fin_pool.tile([128, 512], F32, tag="fin")
                            nc.vector.tensor_copy(fin[:], pf[:])
                            nc.sync.dma_start(
                                out=out[st * 128:(st + 1) * 128, ns],
                                in_=fin[:])
                        chunks.append(emit)
                return chunks

            for qb in range(qb_n):
                q_rhs = [qT_sb[:, h, qb * qb_st:(qb + 1) * qb_st, :]
                         for h in range(HLOC)]
                accA = [acc_pool.tile([128, KG, QCOLS], BF16,
                                      tag=f"accA{h}", name=f"accA{h}")
                        for h in range(HLOC)]
                po = [psP.tile([128, QCOLS], F32, tag=f"po{h}",
                               name=f"po{h}") for h in range(HLOC)]
                ets = dict(ov_ets) if qb == 0 else {}

                def flush(h, g):
                    et = ets.pop((h, g))
                    acc = accA[h][:, :, :]
                    if g == 0:
                        nc.vector.tensor_copy(acc, et[:])
                    else:
                        nc.vector.tensor_add(acc, acc, et[:])
                    for j in range(KG):
                        kt = g * KG + j
                        nc.tensor.matmul(po[h][:],
                                         v_sb[:, kt, h * HD:(h + 1) * HD],
                                         et[:, j, :], start=(kt == 0),
                                         stop=(kt == kt_n - 1))

                for g in range(kg_n):
                    for h in range(HLOC):
                        if (h, g) in ets:
                            continue  # pre-computed in the phase-1 overlap
                        ets[(h, g)] = score_group(psS, h, g, q_rhs[h])
                    if g >= 1:
                        for h in range(HLOC):
                            flush(h, g - 1)
                    if pending:
                        pending.pop(0)()
                    if g % 8 == 7 and pending:
                        pending.pop(0)()
                for h in range(HLOC):
                    flush(h, kg_n - 1)

                # evict po now (DVE) so next qb's PV chains don't wait on the
                # deferred tails; ACT's FIFO stays exps-only
                ats = []
                for h in range(HLOC):
                    araw = araw_pool.tile([128, QCOLS], BF16, tag="araw")
                    nc.vector.tensor_copy(araw[:], po[h][:])
                    at = at_pool.tile([128, QCOLS], BF16, tag="at")
                    ats.append(at)

                    def tail(accA=accA[h], araw=araw, at=at):
                        pdb = psF.tile([128, QCOLS], F32, tag="pf",
                                       name="pdb")
                        pd = pdb[0:1, :]
                        for i in range(KG):
                            nc.tensor.matmul(pd, ones_col_bf[:],
                                             accA[:, i, :],
                                             start=(i == 0),
                                             stop=(i == KG - 1))
                        rsb = rc_pool.tile([1, QCOLS], F32, tag="rsb")
                        nc.vector.reciprocal_approx_fast(rsb[:], pd)
                        rsr = rc_pool.tile([1, QCOLS], BF16, tag="rsr")
                        nc.vector.tensor_copy(rsr[:], rsb[:])
                        nc.tensor.matmul(pdb[:], ones_row[:], rsr[:],
                                         start=True, stop=True)
                        nc.vector.tensor_mul(at[:], araw[:], pdb[:])
                    pending.insert(h, tail)

                pending += outproj_chunks(qb, ats)

            # final drain: the last q-block's tails + out-proj chunks have no
            # later k-loop to interleave into, so the PE runs at ~50% duty
            # here; dummy matmuls keep the HAM activity window busy (a
            # re-throttle would halve the PE clock for the whole drain)
            dum2 = psS.tile([128, KG, QCOLS], F32, tag="sc0", name="dum2")
            ndum = 2 * len(pending)
            di = 0
            while pending:
                pending.pop(0)()
                for _ in range(2):
                    nc.tensor.matmul(dum2[:, 0, :], wdum[:, 0:128], wdum[:],
                                     start=(di == 0), stop=(di == ndum - 1))
                    di += 1


def build_program(s_len=S):
    nc = bacc.Bacc("TRN2", target_bir_lowering=False, debug=False,
                   enable_asserts=False)
    st_n = s_len // 128
    io = {
        "ht": nc.dram_tensor("ht", [st_n, 128, CH, 128], BF16,
                             kind="ExternalInput").ap(),
        "wqkv": nc.dram_tensor("wqkv", [128, CH, NQKV], BF16,
                               kind="ExternalInput").ap(),
        "wo": nc.dram_tensor("wo", [128, HLOC, D], BF16,
                             kind="ExternalInput").ap(),
        "cf": nc.dram_tensor("cf", [s_len, 2, 512], BF16,
                             kind="ExternalInput").ap(),
        "out": nc.dram_tensor("out", [s_len, D], F32,
                              kind="ExternalOutput").ap(),
    }
    with tile.TileContext(nc) as tc:
        build(nc, tc, io, s_len)
    nc.compile()
    return nc


def prep_inputs(inputs, s_len=S):
    """Host-side preprocessing: transposed/tiled bf16 layouts + rope
    coefficient tables (g gains and the 1/sqrt(Hd) scale folded in,
    duplicated per local head and fused q|k for full-width elementwise
    ops)."""
    bf16 = ml_dtypes.bfloat16
    hs = np.asarray(inputs["hidden_states"], np.float32).reshape(s_len, D)
    st_n = s_len // 128
    ht = np.ascontiguousarray(
        hs.reshape(st_n, 128, CH, 128).transpose(0, 3, 2, 1)).astype(bf16)

    fc = np.asarray(inputs["freqs_cis"], np.float32).reshape(s_len, HD)
    cos = np.cos(fc)
    sin = np.sin(fc)
    gq = np.asarray(inputs["gq"], np.float32)
    gk = np.asarray(inputs["gk"], np.float32)

    def coef(g, scale):
        cg = cos * g[None, :] * scale
        sg = np.empty_like(sin)
        sg[:, :64] = -sin[:, :64] * g[None, 64:] * scale
        sg[:, 64:] = sin[:, 64:] * g[None, :64] * scale
        return np.tile(cg, (1, HLOC)), np.tile(sg, (1, HLOC))

    cgq, sgq = coef(gq, SCL)
    cgk, sgk = coef(gk, 1.0)
    cgqk = np.concatenate([cgq, cgk], axis=1)
    sgqk = np.concatenate([sgq, sgk], axis=1)
    cf = np.ascontiguousarray(
        np.stack([cgqk, sgqk], axis=1)).astype(bf16)

    Wq = np.asarray(inputs["Wq"], np.float32)
    Wk = np.asarray(inputs["Wk"], np.float32)
    Wv = np.asarray(inputs["Wv"], np.float32)
    Wo = np.asarray(inputs["Wo"], np.float32)

    in_maps = []
    for c in range(NCORES):
        cols = slice(DLOC * c, DLOC * (c + 1))

        def wtile(W):
            # [D, DLOC] -> [128(part), CH, DLOC]
            return W[cols, :].T.reshape(CH, 128, DLOC).transpose(1, 0, 2)

        wqkv_c = np.ascontiguousarray(
            np.concatenate([wtile(Wq), wtile(Wk), wtile(Wv)],
                           axis=2)).astype(bf16)
        wo_c = np.ascontiguousarray(
            Wo[:, cols].T.reshape(HLOC, 128, D).transpose(1, 0, 2)
        ).astype(bf16)
        in_maps.append({
            "ht": ht, "wqkv": wqkv_c, "wo": wo_c, "cf": cf,
        })
    return in_maps


_CACHE = {}


def run_full(inputs, trace=False, **kw):
    if "nc" not in _CACHE:
        _CACHE["nc"] = build_program(S)
    nc = _CACHE["nc"]
    in_maps = prep_inputs(inputs, S)
    res = bass_utils.run_bass_kernel_spmd(
        nc, in_maps, core_ids=list(range(NCORES)), trace=trace, **kw)
    total = res.results[0]["out"].astype(np.float64)
    for c in range(1, NCORES):
        total += res.results[c]["out"]
    total += np.asarray(inputs["bo"], np.float64)[None, :]
    out = total.astype(np.float32).reshape(1, S, D)
    return out, res


def kernel(**inputs):
    out, _ = run_full(inputs, trace=False)
    return out


# revision 63
# speedup vs baseline: 1.0020x; 1.0016x over previous
"""Trainium2 Bass kernel for ErnieImageAttention (non-causal MHA with per-head
RMSNorm on q/k + rotary embedding), tensor-parallel over heads on 8 NeuronCores.

Sharding: 16 heads / 8 cores = 2 heads per core. Each core computes its heads'
q/k/v projections, attention, and a partial output projection (row-parallel
Wo); the host sums the 8 partials and adds the bias.

Per-core dataflow (S=4096, D=2048, Hd=128, 2 local heads):
  warmup: input DMAs prioritized across the 3 DMA-capable queues (ht0 sliced
    + chunk-0 weights first, wo deferred into the loop); dummy matmuls keep
    the PE HAM activity monitor warm (full 2.4 GHz clock) through the ~12us
    startup DMA wall; ACT tables preload so table switches stay off the
    critical path.
  phase 1 (per 128-row s-tile):
    qkv = hiddenT-chunk matmuls (bf16) into split ppqk/ppv PSUM tiles
    (1-bank each, so 8 banks cover ppqk x3 + ppv + ptp x2 + the overlap's
    score region); RMSNorm stats via Square-with-accum on ACT; RoPE via
    host-precomputed coefficient tables operating q|k fused at full 512-col
    width (the rsqrt scale applied last, per 128-col head block, casting to
    bf16); PE-transpose q/k 128x128 bf16 tiles into [d, s] layout; v [s, d].
  boundary: the first q-block's head-0 score groups + exps are emitted after
    the s-tile loop; they fill the PE while the last tiles' rope chain
    drains on DVE/ACT. Their first Exp is data-gated (zero bias from the
    last tile's stats) behind phase 1's final Sqrt -- Exp and Sqrt live in
    different ACT table sets and each interleave costs a 1.3us reload.
  phase 2 (per 512-col q-block, per head, k in groups of 2 tiles):
    scoresT[k,q] = kT.T @ qT (bf16) into a 2+2 bank PSUM ping-pong per head;
    one wide Exp per group on ACT. Phase 2 is ACT-exp-bound: 256 exps of
    [128,1024] at (N+352)/1.2ns pace the 2.24us/group steady state, with PE
    (scores+PV+out-proj) and DVE (denominator accA adds + evictions) both
    ~98% occupied right behind it.
    denominator: folded into accA lanes by DVE bf16 adds; the deferred tail
    does 2 ones-matmuls + reciprocal + rank-1 bf16 broadcast into a psF bank
    attn_T[d,q] = sum_k V[k,d]^T expT[k,q] accumulated in PSUM (bf16);
    po is evicted to SBUF bf16 by DVE right after the k-loop
  phase 3 (inline per q-block): fin[s, :2048] = sum_h attnT_h.T @ WoT_h, all
    bf16 (fp32 matmuls run ~2.7x slower on PE); psF has 2 rotating banks so
    out-proj chunks pipeline instead of serializing on one bank; the final
    drain interleaves dummy matmuls so the HAM never rethrottles the clock.

Softmax is max-subtraction-free: logits are ~N(0,1) by construction
(RMSNorm'd q/k, 1/sqrt(Hd) folded into q's rope tables).
"""

import numpy as np
import ml_dtypes

import concourse.bass as bass
import concourse.tile as tile
from concourse import bacc, mybir
from concourse import bass_utils
from concourse.masks import make_identity

F32 = mybir.dt.float32
F32R = mybir.dt.float32r
BF16 = mybir.dt.bfloat16
AX = mybir.AxisListType
AF = mybir.ActivationFunctionType

S = 4096
D = 2048
HD = 128
HEADS = 16
NCORES = 8
HLOC = HEADS // NCORES  # 2 heads per core
DLOC = HLOC * HD  # 256 local head dims
NQKV = 3 * DLOC  # fused q|k|v projection width
CH = D // 128  # 16 contraction chunks for projections
EPS = 1e-5
SCL = 1.0 / np.sqrt(HD)

QCOLS = 512  # q columns per attention block
NWARM0 = 10  # startup HAM-warming dummy matmuls
NWARM1 = 14  # post-tile-0 dummies bridging the tail of the weight-DMA wall
NWARMB = 10  # dummies filling the last tile's rope-wait before phase 2
NOVERLAP = 4  # qb0/h0 score groups overlapped into the phase-1 tail


def build(nc, tc, io, s_len):
    st_n = s_len // 128  # s tiles
    qb_n = s_len // QCOLS  # q blocks
    qb_st = QCOLS // 128  # s tiles per q block
    kt_n = st_n  # k tiles

    ht, wqkv, wo, cf, out = (
        io["ht"], io["wqkv"], io["wo"], io["cf"], io["out"],
    )

    import contextlib

    with contextlib.ExitStack() as ctx:
        ctx.enter_context(nc.allow_low_precision(
            reason="bf16/f32r compute; values are O(1) and the rel-err "
                   "budget is 2e-2"))
        consts = ctx.enter_context(tc.tile_pool(name="consts", bufs=1))
        persist = ctx.enter_context(tc.tile_pool(name="persist", bufs=1))
        ht_pool = ctx.enter_context(tc.tile_pool(name="ht", bufs=3))
        cs_pool = ctx.enter_context(tc.tile_pool(name="cs", bufs=3))
        work = ctx.enter_context(tc.tile_pool(name="work", bufs=2))
        et_pool = ctx.enter_context(tc.tile_pool(name="et", bufs=10))
        at_pool = ctx.enter_context(tc.tile_pool(name="at", bufs=6))
        araw_pool = ctx.enter_context(tc.tile_pool(name="araw", bufs=3))
        acc_pool = ctx.enter_context(tc.tile_pool(name="acc", bufs=2))
        rc_pool = ctx.enter_context(tc.tile_pool(name="rc", bufs=2))
        fin_pool = ctx.enter_context(tc.tile_pool(name="fin", bufs=6))

        # startup DMAs first on their queues, prioritized so the first real
        # matmul's inputs (ht0 chunk 0 + weight chunk 0) finish first: ht0
        # arrives in 4 slices on the SP ring, the weight parts fan out over
        # the gpsimd/scalar rings, and wo (not needed until the first
        # out-projection at ~40% of the kernel) is deferred into the loop.
        ht0_t = ht_pool.tile([128, CH, 128], BF16, tag="ht", name="ht0")
        for q in range(2):
            nc.sync.dma_start(out=ht0_t[:, 8 * q:8 * (q + 1), :],
                              in_=ht[0][:, 8 * q:8 * (q + 1), :])
        cf0_t = cs_pool.tile([128, 2, 512], BF16, tag="cf", name="cf0")
        nc.scalar.dma_start(out=cf0_t[:], in_=cf[0:128, :, :])

        w_parts = [(0, 1, nc.gpsimd), (1, 5, nc.gpsimd), (6, 5, nc.scalar),
                   (11, 5, nc.sync)]
        w_sb = [None] * CH  # per-chunk views
        for pi, (c0, ln, eng) in enumerate(w_parts):
            t = consts.tile([128, ln, NQKV], BF16, name=f"wqkv{pi}",
                            tag=f"wqkv{pi}")
            eng.dma_start(out=t[:], in_=wqkv[:, c0:c0 + ln, :])
            for j in range(ln):
                w_sb[c0 + j] = t[:, j, :]
        wo_sb = consts.tile([128, HLOC, D], BF16)

        # constants (warmup scratch first so the dummy matmuls start early)
        wdum = consts.tile([128, 512], BF16)
        nc.vector.memset(wdum[:], 0.0)
        ident = consts.tile([128, 128], BF16)
        make_identity(nc, ident[:])
        ones_f32 = consts.tile([128, 1], F32)
        nc.vector.memset(ones_f32[:], 1.0)
        ones_col_bf = consts.tile([128, 1], BF16)
        nc.vector.tensor_copy(ones_col_bf[:], ones_f32[:])
        ones_row_f32 = consts.tile([1, 128], F32)
        nc.vector.memset(ones_row_f32[:], 1.0)
        ones_row = consts.tile([1, 128], BF16)
        nc.vector.tensor_copy(ones_row[:], ones_row_f32[:])
        eps_t = consts.tile([128, 1], F32)
        nc.vector.memset(eps_t[:], EPS)

        # ACT table preloads (exp first so its set sits in slot 0 for all of
        # phase 2; sqrt's set lands in slot 1 for phase 1)
        wsc = consts.tile([128, 1], F32)
        nc.vector.memset(wsc[:], 1.0)
        wout = consts.tile([128, 1], F32)
        nc.scalar.activation(wout[:], wsc[:], AF.Exp)
        nc.scalar.activation(wout[:], wsc[:], AF.Square)
        nc.scalar.activation(wout[:], wsc[:], AF.Sqrt, bias=eps_t[:],
                             scale=1.0 / HD)
        nc.scalar.copy(wout[:], wsc[:])

        # persistent per-head transposed q/k and v
        qT_sb = persist.tile([128, HLOC, st_n, 128], BF16)
        kT_sb = persist.tile([128, HLOC, st_n, 128], BF16)
        v_sb = persist.tile([128, st_n, DLOC], BF16)

        KG = 2  # k tiles per score group
        kg_n = kt_n // KG
        ov_ets = {}  # (h, g) -> et tile, prepopulated by the overlap

        def score_group(pool, h, g, q_rhs_h, bias=0.0, split_exp=False):
            sc = pool.tile([128, KG, QCOLS], F32, tag=f"sc{h}",
                           name=f"sc{h}")
            for j in range(KG):
                nc.tensor.matmul(sc[:, j, :], kT_sb[:, h, g * KG + j, :],
                                 q_rhs_h, start=True, stop=True)
            et = et_pool.tile([128, KG, QCOLS], BF16, tag="et")
            if split_exp:
                # per-k-tile exps: subtile WAR lets the next group's first
                # matmul start as soon as the matching half is consumed
                for j in range(KG):
                    nc.scalar.activation(et[:, j, :], sc[:, j, :], AF.Exp,
                                         bias=bias)
            else:
                nc.scalar.activation(et[:], sc[:], AF.Exp, bias=bias)
            return et

        # ---------------- phase 1: projections + norm + rope + transpose ----
        # psS0 holds one single-k-tile score bank so the first q-block's
        # score matmuls can overlap the phase-1 tail (the PE would otherwise
        # idle while the last s-tiles' rope drains on DVE); it doubles as the
        # target for HAM-warming dummy matmuls during the startup DMA wait.
        with (
            tc.tile_pool(name="psS0", bufs=1, space="PSUM") as psS0,
            tc.tile_pool(name="ps1", bufs=1, space="PSUM") as ps1,
        ):
            dum = psS0.tile([128, KG, QCOLS], F32, tag="sc0", name="dum")
            for i in range(NWARM0):
                nc.tensor.matmul(dum[:, 0, :], wdum[:, 0:128], wdum[:],
                                 start=(i == 0), stop=False)

            for st in range(st_n):
                if st == 0:
                    ht_t = ht0_t
                    cf_t = cf0_t
                else:
                    ss = slice(st * 128, (st + 1) * 128)
                    ht_t = ht_pool.tile([128, CH, 128], BF16, tag="ht")
                    nc.sync.dma_start(out=ht_t[:], in_=ht[st])
                    cf_t = cs_pool.tile([128, 2, 512], BF16, tag="cf")
                    nc.sync.dma_start(out=cf_t[:], in_=cf[ss, :, :])
                if st == 6:
                    # wo fetch deferred off the startup critical path
                    nc.scalar.dma_start(out=wo_sb[:], in_=wo)

                ppqk = ps1.tile([128, 512], F32, tag="ppqk", bufs=3)
                ppv = ps1.tile([128, 256], F32, tag="ppv", bufs=1)
                for c in range(CH):
                    lhs = ht_t[:, c, :]
                    # moving operand limit is 512 elements: q|k then v
                    nc.tensor.matmul(ppqk[:], lhs, w_sb[c][:, 0:512],
                                     start=(c == 0), stop=(c == CH - 1))
                    nc.tensor.matmul(ppv[:], lhs, w_sb[c][:, 512:NQKV],
                                     start=(c == 0), stop=(c == CH - 1))
                    if st == 0:
                        # tile 0 is weight-DMA-supply-limited: dummy matmuls
                        # between chunks keep the PE HAM busy-window alive
                        nc.tensor.matmul(dum[:, 0, :], wdum[:, 0:128],
                                         wdum[:], start=False, stop=False)

                if st == 0:
                    # ~5us of DMA wall remains after tile 0; bridge most of
                    # it (under-budgeted so real work is never displaced)
                    for i in range(NWARM1):
                        nc.tensor.matmul(dum[:, 0, :], wdum[:, 0:128],
                                         wdum[:], start=False,
                                         stop=(i == NWARM1 - 1))
                if st == st_n - 1:
                    # the last tile's transposes sit next in the in-order PE
                    # queue but wait ~4us for its rope chain on DVE; these
                    # fill that window (and keep the HAM clock warm into the
                    # phase boundary)
                    dumB = psS0.tile([128, KG, QCOLS], F32, tag="sc0",
                                     name="dumB")
                    for i in range(NWARMB):
                        nc.tensor.matmul(dumB[:, 0, :], wdum[:, 0:128],
                                         wdum[:], start=(i == 0),
                                         stop=(i == NWARMB - 1))

                # v: PSUM -> SBUF bf16 (first ACT op of the tile so the
                # single-buffered ppv bank frees before the next tile)
                nc.scalar.copy(v_sb[:, st, :], ppv[:])

                # rms stats: Square with free-dim accumulation -> sum(x^2)
                # per 128-col block (q0, q1, k0, k1)
                var4 = work.tile([128, 4], F32, tag="var4")
                sqd = work.tile([128, HD], F32, tag="sqd")  # dump
                for j in range(4):
                    nc.scalar.activation(sqd[:],
                                         ppqk[:, j * 128:(j + 1) * 128],
                                         AF.Square,
                                         accum_out=var4[:, j:j + 1])
                sig4 = work.tile([128, 4], F32, tag="sig4")
                nc.scalar.activation(sig4[:], var4[:], AF.Sqrt,
                                     bias=eps_t[:], scale=1.0 / HD)
                r4 = work.tile([128, 4], F32, tag="r4")
                nc.vector.reciprocal_approx_fast(r4[:], sig4[:])

                # rope on the fused q|k 512-col block:
                #   xf = x . CG + shift64(x) . SG ; xa = r * xf (per head blk)
                pqk = ppqk[:]
                m1 = work.tile([128, 512], F32, tag="m1")
                nc.vector.tensor_mul(m1[:], pqk, cf_t[:, 0, :])
                m2 = work.tile([128, 512], F32, tag="m2")
                x4 = pqk.rearrange("p (h t u) -> p h t u", h=4, t=2)
                m4 = m2[:].rearrange("p (h t u) -> p h t u", h=4, t=2)
                g4 = cf_t[:, 1, :].rearrange("p (h t u) -> p h t u", h=4, t=2)
                nc.vector.tensor_mul(m4[:, :, 0, :], x4[:, :, 1, :],
                                     g4[:, :, 0, :])
                nc.vector.tensor_mul(m4[:, :, 1, :], x4[:, :, 0, :],
                                     g4[:, :, 1, :])
                xf = work.tile([128, 512], F32, tag="xf")
                nc.vector.tensor_add(xf[:], m1[:], m2[:])
                xa = work.tile([128, 512], BF16, tag="xa")
                for j in range(4):
                    js = slice(j * 128, (j + 1) * 128)
                    nc.vector.tensor_scalar_mul(xa[:, js], xf[:, js],
                                                r4[:, j:j + 1])
                # blocks: 0,1 -> qT heads; 2,3 -> kT heads. (PE transpose +
                # ACT eviction; DMA XBAR transposes measured far slower --
                # ~1.2us queue time per trigger plus a slow transfer path.)
                # The LAST tile's transposes are deferred into the qb0 loop:
                # in the in-order PE queue they would stall all of phase 2
                # behind this tile's ~4us rope latency, yet their outputs
                # aren't consumed until deep into the first q-block.
                if st == st_n - 1:
                    deferred_tp = xa
                else:
                    for j in range(4):
                        dstT, h = (qT_sb, j) if j < 2 else (kT_sb, j - 2)
                        ptp = ps1.tile([128, 128], BF16, tag="ptp", bufs=2)
                        nc.tensor.transpose(ptp[:],
                                            xa[:, j * 128:(j + 1) * 128],
                                            ident[:])
                        nc.scalar.copy(dstT[:, h, st, :], ptp[:])

            # overlap: the first q-block's head-0 score groups fill the PE
            # while the last s-tiles' rope chain drains on DVE/ACT. The
            # first exp carries a zero bias derived from the last tile's r4
            # so the scheduler cannot hoist it before phase 1's final Sqrt
            # (Exp and Sqrt live in different ACT table sets; an interleave
            # costs a 1.3us table reload per switch).
            zb = work.tile([128, 1], F32, tag="zb")
            nc.vector.tensor_scalar_mul(zb[:], sig4[:, 0:1], 0.0)
            q0 = qT_sb[:, 0, 0:qb_st, :]
            for g in range(NOVERLAP):
                ov_ets[(0, g)] = score_group(psS0, 0, g, q0,
                                             bias=zb[:] if g == 0 else 0.0,
                                             split_exp=True)

        # ---------------- phase 2+3: attention + output projection ----------
        # The two heads' score->exp->PV chains are interleaved step by step:
        # while ACT runs one head's exp, PE runs the other head's matmuls, so
        # every cross-engine dependency has a full step (~2.3us) of slack and
        # semaphore latency never stalls the in-order PE queue.
        # PSUM: sc_h0(2) + sc_h1(2) + po_h0(1) + po_h1(1) + pf(2, shared with
        # the tails' pd/broadcast bank) = 8.
        with (
            tc.tile_pool(name="psS", bufs=1, space="PSUM") as psS,
            tc.tile_pool(name="psP", bufs=1, space="PSUM") as psP,
            tc.tile_pool(name="psF", bufs=2, space="PSUM") as psF,
        ):
            pending = []  # deferred tails + out-proj chunks

            def outproj_chunks(qb, ats):
                chunks = []
                last = qb == qb_n - 1
                for sti in range(qb_st):
                    st = qb * qb_st + sti
                    sl = slice(sti * 128, (sti + 1) * 128)
                    for nchunk in range(D // 512):
                        ns = slice(nchunk * 512, (nchunk + 1) * 512)
                        # the last q-block's chunks drain with no k-loop to
                        # hide behind and the fin eviction is the pacer;
                        # ACT is exp-free there, so alternate the copies
                        # (Copy is filler in the exp table set -- no reload)
                        act = last and nchunk % 2 == 0

                        def emit(st=st, sl=sl, ns=ns, ats=ats, act=act):
                            pf = psF.tile([128, QCOLS], F32, tag="pf",
                                          name=f"pf_{st}_{ns.start}")
                            for h in range(HLOC):
                                nc.tensor.matmul(pf[:], ats[h][:, sl],
                                                 wo_sb[:, h, ns],
                                                 start=(h == 0),
                                                 stop=(h == HLOC - 1))
                            fin = fin_pool.tile([128, 512], F32, tag="fin")
                            if act:
                                nc.scalar.copy(fin[:], pf[:])
                            else:
                                nc.vector.tensor_copy(fin[:], pf[:])
                            nc.sync.dma_start(
                                out=out[st * 128:(st + 1) * 128, ns],
                                in_=fin[:])
                        chunks.append(emit)
                return chunks

            for qb in range(qb_n):
                q_rhs = [qT_sb[:, h, qb * qb_st:(qb + 1) * qb_st, :]
                         for h in range(HLOC)]
                accA = [acc_pool.tile([128, KG, QCOLS], BF16,
                                      tag=f"accA{h}", name=f"accA{h}")
                        for h in range(HLOC)]
                po = [psP.tile([128, QCOLS], F32, tag=f"po{h}",
                               name=f"po{h}") for h in range(HLOC)]
                ets = dict(ov_ets) if qb == 0 else {}

                def flush(h, g):
                    et = ets.pop((h, g))
                    acc = accA[h][:, :, :]
                    if g == 0:
                        nc.vector.tensor_copy(acc, et[:])
                    else:
                        nc.vector.tensor_add(acc, acc, et[:])
                    for j in range(KG):
                        kt = g * KG + j
                        nc.tensor.matmul(po[h][:],
                                         v_sb[:, kt, h * HD:(h + 1) * HD],
                                         et[:, j, :], start=(kt == 0),
                                         stop=(kt == kt_n - 1))

                for g in range(kg_n):
                    for h in range(HLOC):
                        if (h, g) in ets:
                            continue  # pre-computed in the phase-1 overlap
                        ets[(h, g)] = score_group(psS, h, g, q_rhs[h])
                    if g >= 1:
                        for h in range(HLOC):
                            flush(h, g - 1)
                    if pending:
                        pending.pop(0)()
                    if g % 8 == 7 and pending:
                        pending.pop(0)()
                    if qb == 0 and g == 2:
                        # the last s-tile's deferred transposes: its rope
                        # finished during the overlap groups, and kT[...,
                        # st_n-1] isn't read until group kg_n-1
                        for j in range(4):
                            dstT, h = ((qT_sb, j) if j < 2
                                       else (kT_sb, j - 2))
                            ptp2 = psF.tile([128, 128], BF16, tag="pf",
                                            name="ptp2")
                            nc.tensor.transpose(
                                ptp2[:],
                                deferred_tp[:, j * 128:(j + 1) * 128],
                                ident[:])
                            nc.vector.tensor_copy(dstT[:, h, st_n - 1, :],
                                                  ptp2[:])
                for h in range(HLOC):
                    flush(h, kg_n - 1)

                # evict po now (DVE) so next qb's PV chains don't wait on the
                # deferred tails; ACT's FIFO stays exps-only
                ats = []
                for h in range(HLOC):
                    araw = araw_pool.tile([128, QCOLS], BF16, tag="araw")
                    nc.vector.tensor_copy(araw[:], po[h][:])
                    at = at_pool.tile([128, QCOLS], BF16, tag="at")
                    ats.append(at)

                    def tail(accA=accA[h], araw=araw, at=at):
                        pdb = psF.tile([128, QCOLS], F32, tag="pf",
                                       name="pdb")
                        pd = pdb[0:1, :]
                        for i in range(KG):
                            nc.tensor.matmul(pd, ones_col_bf[:],
                                             accA[:, i, :],
                                             start=(i == 0),
                                             stop=(i == KG - 1))
                        rsb = rc_pool.tile([1, QCOLS], F32, tag="rsb")
                        nc.vector.reciprocal_approx_fast(rsb[:], pd)
                        rsr = rc_pool.tile([1, QCOLS], BF16, tag="rsr")
                        nc.vector.tensor_copy(rsr[:], rsb[:])
                        nc.tensor.matmul(pdb[:], ones_row[:], rsr[:],
                                         start=True, stop=True)
                        nc.vector.tensor_mul(at[:], araw[:], pdb[:])
                    pending.insert(h, tail)

                pending += outproj_chunks(qb, ats)

            # final drain: the last q-block's tails + out-proj chunks have no
            # later k-loop to interleave into, so the PE runs at ~50% duty
            # here; dummy matmuls keep the HAM activity window busy (a
            # re-throttle would halve the PE clock for the whole drain)
            dum2 = psS.tile([128, KG, QCOLS], F32, tag="sc0", name="dum2")
            ndum = 2 * len(pending)
            di = 0
            while pending:
                pending.pop(0)()
                for _ in range(2):
                    nc.tensor.matmul(dum2[:, 0, :], wdum[:, 0:128], wdum[:],
                                     start=(di == 0), stop=(di == ndum - 1))
                    di += 1


def build_program(s_len=S):
    nc = bacc.Bacc("TRN2", target_bir_lowering=False, debug=False,
                   enable_asserts=False)
    st_n = s_len // 128
    io = {
        "ht": nc.dram_tensor("ht", [st_n, 128, CH, 128], BF16,
                             kind="ExternalInput").ap(),
        "wqkv": nc.dram_tensor("wqkv", [128, CH, NQKV], BF16,
                               kind="ExternalInput").ap(),
        "wo": nc.dram_tensor("wo", [128, HLOC, D], BF16,
                             kind="ExternalInput").ap(),
        "cf": nc.dram_tensor("cf", [s_len, 2, 512], BF16,
                             kind="ExternalInput").ap(),
        "out": nc.dram_tensor("out", [s_len, D], F32,
                              kind="ExternalOutput").ap(),
    }
    with tile.TileContext(nc) as tc:
        build(nc, tc, io, s_len)
    nc.compile()
    return nc


def prep_inputs(inputs, s_len=S):
    """Host-side preprocessing: transposed/tiled bf16 layouts + rope
    coefficient tables (g gains and the 1/sqrt(Hd) scale folded in,
    duplicated per local head and fused q|k for full-width elementwise
    ops)."""
    bf16 = ml_dtypes.bfloat16
    hs = np.asarray(inputs["hidden_states"], np.float32).reshape(s_len, D)
    st_n = s_len // 128
    ht = np.ascontiguousarray(
        hs.reshape(st_n, 128, CH, 128).transpose(0, 3, 2, 1)).astype(bf16)

    fc = np.asarray(inputs["freqs_cis"], np.float32).reshape(s_len, HD)
    cos = np.cos(fc)
    sin = np.sin(fc)
    gq = np.asarray(inputs["gq"], np.float32)
    gk = np.asarray(inputs["gk"], np.float32)

    def coef(g, scale):
        cg = cos * g[None, :] * scale
        sg = np.empty_like(sin)
        sg[:, :64] = -sin[:, :64] * g[None, 64:] * scale
        sg[:, 64:] = sin[:, 64:] * g[None, :64] * scale
        return np.tile(cg, (1, HLOC)), np.tile(sg, (1, HLOC))

    cgq, sgq = coef(gq, SCL)
    cgk, sgk = coef(gk, 1.0)
    cgqk = np.concatenate([cgq, cgk], axis=1)
    sgqk = np.concatenate([sgq, sgk], axis=1)
    cf = np.ascontiguousarray(
        np.stack([cgqk, sgqk], axis=1)).astype(bf16)

    Wq = np.asarray(inputs["Wq"], np.float32)
    Wk = np.asarray(inputs["Wk"], np.float32)
    Wv = np.asarray(inputs["Wv"], np.float32)
    Wo = np.asarray(inputs["Wo"], np.float32)

    in_maps = []
    for c in range(NCORES):
        cols = slice(DLOC * c, DLOC * (c + 1))

        def wtile(W):
            # [D, DLOC] -> [128(part), CH, DLOC]
            return W[cols, :].T.reshape(CH, 128, DLOC).transpose(1, 0, 2)

        wqkv_c = np.ascontiguousarray(
            np.concatenate([wtile(Wq), wtile(Wk), wtile(Wv)],
                           axis=2)).astype(bf16)
        wo_c = np.ascontiguousarray(
            Wo[:, cols].T.reshape(HLOC, 128, D).transpose(1, 0, 2)
        ).astype(bf16)
        in_maps.append({
            "ht": ht, "wqkv": wqkv_c, "wo": wo_c, "cf": cf,
        })
    return in_maps


_CACHE = {}


def run_full(inputs, trace=False, **kw):
    if "nc" not in _CACHE:
        _CACHE["nc"] = build_program(S)
    nc = _CACHE["nc"]
    in_maps = prep_inputs(inputs, S)
    res = bass_utils.run_bass_kernel_spmd(
        nc, in_maps, core_ids=list(range(NCORES)), trace=trace, **kw)
    total = res.results[0]["out"].astype(np.float64)
    for c in range(1, NCORES):
        total += res.results[c]["out"]
    total += np.asarray(inputs["bo"], np.float64)[None, :]
    out = total.astype(np.float32).reshape(1, S, D)
    return out, res


def kernel(**inputs):
    out, _ = run_full(inputs, trace=False)
    return out
